# revision 3
# baseline (speedup 1.0000x reference)
"""Self-contained Trainium2 Bass kernel for the attention-like module:

    Q = x @ Wq.T + bq;  K = x @ Wk.T + bk;  V = x @ Wv.T + bv
    S = Q.T @ K;  A = softmax(S / sqrt(dk), axis=0);  out = V @ A

Algorithm (Gram-matrix restructure; N=8192 rows sharded over 8 cores):

  S = Wq Cxx Wk^T + u bk^T + bq v^T + N bq bk^T
      with Cxx = x^T x (AllReduced), m = x^T 1, u = Wq m, v = Wk m.
  Each core owns a 128-column shard Jc of S: it computes
  A1 = Cxx Wk^T[:,Jc], S[:,Jc] = Wq A1 + rank-1 terms, E = exp(S/32),
  den_j = sum_a E[a,j], then the output reduces to
      out = x @ W2 + 1_N r,  W2 = Wv^T E diag(1/den), r = bv^T E diag(1/den)
  W2's column shards are AllGathered (tiny: 256KB/core) and each core
  computes its row-shard of out = x @ W2 + r.

Per-core tensor work ~5.1 GF (vs 10.7 GF for the direct algorithm);
collectives: 2-way-split bf16 AllReduce of Cxx (2MB) + bf16 AllGather (2MB).
"""

import numpy as np
import ml_dtypes

N, D, NCORES, P, F = 8192, 1024, 8, 128, 512
NPC = N // NCORES   # rows per core (1024)
KC = D // P         # 128-chunks of the model dim (8)
JW = D // NCORES    # column shard width per core (128)
NORM = 1.0 / float(np.sqrt(D))

_cache = {}


def _build_nc():
    import concourse.mybir as mybir
    import concourse.tile as tile
    from concourse import bacc

    f32 = mybir.dt.float32
    bf16 = mybir.dt.bfloat16
    add = mybir.AluOpType.add
    mult = mybir.AluOpType.mult
    Exp = mybir.ActivationFunctionType.Exp
    RG = [list(range(NCORES))]

    nc = bacc.Bacc("TRN2", target_bir_lowering=False, debug=False,
                   num_devices=NCORES)

    xn = nc.dram_tensor("xn", [NPC, D], bf16, kind="ExternalInput").ap()
    xt = nc.dram_tensor("xt", [D, NPC], bf16, kind="ExternalInput").ap()
    wqT = nc.dram_tensor("wqT", [D, D], bf16, kind="ExternalInput").ap()
    wvn = nc.dram_tensor("wvn", [D, D], bf16, kind="ExternalInput").ap()
    wkj = nc.dram_tensor("wkj", [D, JW], bf16, kind="ExternalInput").ap()
    bq_r = nc.dram_tensor("bq_r", [1, D], bf16, kind="ExternalInput").ap()
    bk_r = nc.dram_tensor("bk_r", [1, JW], bf16, kind="ExternalInput").ap()
    bkN_r = nc.dram_tensor("bkN_r", [1, JW], f32, kind="ExternalInput").ap()
    bv_c = nc.dram_tensor("bv_c", [P, KC], bf16, kind="ExternalInput").ap()
    out = nc.dram_tensor("out", [NPC, D], f32, kind="ExternalOutput").ap()

    with tile.TileContext(nc) as tc:
        with tc.tile_pool(name="persist", bufs=1) as pp, \
             tc.tile_pool(name="stage", bufs=4) as sp, \
             tc.tile_pool(name="psA", bufs=4, space="PSUM") as psA, \
             tc.tile_pool(name="psB", bufs=2, space="PSUM") as psB, \
             tc.tile_pool(name="psC", bufs=2, space="PSUM") as psC, \
             tc.tile_pool(name="dram", bufs=1, space="DRAM") as dp:

            # ---- resident inputs (chunked DMAs; x first: Cxx needs it) ----
            xs = pp.tile([P, KC, NPC], bf16, name="xs")
            xnr = xn.rearrange("(c p) d -> p c d", p=P)
            for c in range(KC):
                nc.sync.dma_start(xs[:, c], xnr[:, c])
            wq = pp.tile([P, KC, D], bf16, name="wq")
            wqr = wqT.rearrange("(c p) i -> p c i", p=P)
            wkjs = pp.tile([P, KC, JW], bf16, name="wkjs")
            wkjr = wkj.rearrange("(c p) j -> p c j", p=P)
            nc.sync.dma_start(wkjs[:], wkjr[:])
            for c in range(KC):
                nc.sync.dma_start(wq[:, c], wqr[:, c])
            wvs = pp.tile([P, KC, D], bf16, name="wvs")
            wvr = wvn.rearrange("(c p) d -> p c d", p=P)
            for c in range(KC):
                nc.sync.dma_start(wvs[:, c], wvr[:, c])
            xts = pp.tile([P, KC, NPC], bf16, name="xts")
            xtr = xt.rearrange("(c p) n -> p c n", p=P)
            for c in range(KC):
                nc.sync.dma_start(xts[:, c], xtr[:, c])
            bqs = pp.tile([1, D], bf16, name="bqs")
            nc.sync.dma_start(bqs[:], bq_r[:])
            bks = pp.tile([1, JW], bf16, name="bks")
            nc.sync.dma_start(bks[:], bk_r[:])
            bkNs = pp.tile([1, JW], f32, name="bkNs")
            nc.sync.dma_start(bkNs[:], bkN_r[:])
            bvs = pp.tile([P, KC], bf16, name="bvs")
            nc.sync.dma_start(bvs[:], bv_c[:])
            ones_c = pp.tile([P, 1], bf16, name="ones_c")
            nc.any.memset(ones_c[:], 1.0)
            ones_rf = pp.tile([1, P], f32, name="ones_rf")
            nc.any.memset(ones_rf[:], 1.0)
            ones_rb = pp.tile([1, P], bf16, name="ones_rb")
            nc.any.memset(ones_rb[:], 1.0)

            # ---- collective buffers ----
            cxx_b0 = dp.tile([D // 2, D], bf16, name="cxx_b0")
            cxx_r0 = dp.tile([D // 2, D], bf16, name="cxx_r0",
                             addr_space="Shared")
            cxx_b1 = dp.tile([D // 2 + 1, D], bf16, name="cxx_b1")
            cxx_r1 = dp.tile([D // 2 + 1, D], bf16, name="cxx_r1",
                             addr_space="Shared")
            ag_in = dp.tile([D + 1, JW], bf16, name="ag_in")
            ag_out = dp.tile([NCORES, D + 1, JW], bf16, name="ag_out",
                             addr_space="Shared")

            # ---- phase 1: Cxx = x^T x (chunked) and m = x^T 1 ----
            m_ps = psC.tile([1 * P, F], f32, tag="psC", name="m_ps")
            for d1c in range(KC):
                ph = [psA.tile([P, F], f32, tag="psA", name=f"cxx{h}")
                      for h in range(2)]
                for nch in range(KC):
                    lhs = xs[:, nch, d1c * P:(d1c + 1) * P]
                    st, sp_ = (nch == 0), (nch == KC - 1)
                    nc.tensor.matmul(ph[0][:], lhs, xs[:, nch, 0:F],
                                     start=st, stop=sp_)
                    nc.tensor.matmul(ph[1][:], lhs, xs[:, nch, F:D],
                                     start=st, stop=sp_)
                    nc.tensor.matmul(m_ps[:, d1c:d1c + 1], lhs, ones_c[:],
                                     start=(st and d1c == 0),
                                     stop=(sp_ and d1c == KC - 1))
                dst = cxx_b0 if d1c < 4 else cxx_b1
                for h in range(2):
                    cst = sp.tile([P, F], bf16, tag="cxst", name="cxst")
                    nc.vector.tensor_copy(cst[:], ph[h][:])
                    nc.sync.dma_start(
                        dst[(d1c % 4) * P:(d1c % 4 + 1) * P, h * F:(h + 1) * F],
                        cst[:])
                if d1c == 3:
                    nc.gpsimd.collective_compute(
                        "AllReduce", add, replica_groups=RG,
                        ins=[cxx_b0.opt()], outs=[cxx_r0.opt()])
            m_sb = pp.tile([P, KC], bf16, name="m_sb")
            nc.vector.tensor_copy(m_sb[:], m_ps[:, 0:KC])
            nc.sync.dma_start(
                cxx_b1[D // 2:D // 2 + 1, :].rearrange(
                    "o (p c) -> (o p) c", p=P), m_sb[:])
            nc.gpsimd.collective_compute(
                "AllReduce", add, replica_groups=RG,
                ins=[cxx_b1.opt()], outs=[cxx_r1.opt()])

            # ---- phase 2: A1 = Cxx @ WkT[:,Jc]; u = Wq m; v = Wk m ----
            cxx_sb = pp.tile([P, KC, D], bf16, name="cxx_sb")
            for d2c in range(KC):
                src_ = cxx_r0 if d2c < 4 else cxx_r1
                nc.sync.dma_start(cxx_sb[:, d2c],
                                  src_[(d2c % 4) * P:(d2c % 4 + 1) * P, :])
            a1sb = pp.tile([P, KC, P], bf16, name="a1sb")
            a1a = pp.tile([P, KC, P], f32, name="a1a")
            for d1c in range(KC):
                a1t = psB.tile([P, F], f32, tag="psB", name="a1t")
                for d2c in range(4):
                    nc.tensor.matmul(a1t[:, 0:P],
                                     cxx_sb[:, d2c, d1c * P:(d1c + 1) * P],
                                     wkjs[:, d2c, :],
                                     start=(d2c == 0), stop=(d2c == 3))
                nc.vector.tensor_copy(a1a[:, d1c, :], a1t[:, 0:P])
            u_row = pp.tile([1, D], bf16, name="u_row")
            w_row = pp.tile([1, JW], bf16, name="w_row")
            mcol = pp.tile([P, KC], bf16, name="mcol")
            nc.sync.dma_start(
                mcol[:], cxx_r1[D // 2:D // 2 + 1, :].rearrange(
                    "o (p c) -> (o p) c", p=P))
            for h in range(2):
                ups = psC.tile([1, F], f32, tag="psC", name="ups")
                for dc in range(KC):
                    nc.tensor.matmul(ups[:], mcol[:, dc:dc + 1],
                                     wq[:, dc, h * F:(h + 1) * F],
                                     start=(dc == 0), stop=(dc == KC - 1))
                nc.vector.tensor_copy(u_row[:, h * F:(h + 1) * F], ups[:])
            vps = psC.tile([1, F], f32, tag="psC", name="vps")
            for dc in range(KC):
                nc.tensor.matmul(vps[:, 0:JW], mcol[:, dc:dc + 1],
                                 wkjs[:, dc, :],
                                 start=(dc == 0), stop=(dc == KC - 1))
            nc.vector.tensor_tensor(w_row[:], vps[:, 0:JW], bkNs[:], add)
            for d1c in range(KC):
                a1t = psB.tile([P, F], f32, tag="psB", name="a1t")
                for d2c in range(4, KC):
                    nc.tensor.matmul(a1t[:, 0:P],
                                     cxx_sb[:, d2c, d1c * P:(d1c + 1) * P],
                                     wkjs[:, d2c, :],
                                     start=(d2c == 4), stop=(d2c == KC - 1))
                nc.vector.tensor_tensor(a1sb[:, d1c, :], a1a[:, d1c, :],
                                        a1t[:, 0:P], add)

            # ---- S = Wq A1 + u bk^T + bq (v + N bk)^T; E = exp(S/32) ----
            e_sb = pp.tile([P, KC, JW], bf16, name="e_sb")
            s_st = pp.tile([P, KC, JW], f32, name="s_st")
            for ic in range(KC):
                sps = psB.tile([P, F], f32, tag="psB", name="sps")
                for dc in range(KC):
                    nc.tensor.matmul(sps[:, 0:JW],
                                     wq[:, dc, ic * P:(ic + 1) * P],
                                     a1sb[:, dc, :],
                                     start=(dc == 0), stop=False)
                nc.tensor.matmul(sps[:, 0:JW], u_row[:, ic * P:(ic + 1) * P],
                                 bks[:], start=False, stop=False)
                nc.tensor.matmul(sps[:, 0:JW], bqs[:, ic * P:(ic + 1) * P],
                                 w_row[:], start=False, stop=True)
                nc.vector.tensor_copy(s_st[:, ic, :], sps[:, 0:JW])
            for h2 in range(2):
                nc.scalar.activation(
                    e_sb[:, h2 * 4:(h2 + 1) * 4, :].rearrange(
                        "p a b -> p (a b)"),
                    s_st[:, h2 * 4:(h2 + 1) * 4, :].rearrange(
                        "p a b -> p (a b)"),
                    Exp, scale=NORM)

            # ---- den, 1/den broadcast ----
            dps = psC.tile([1, F], f32, tag="psC", name="dps")
            for ac in range(KC):
                nc.tensor.matmul(dps[:, 0:JW], ones_c[:], e_sb[:, ac, :],
                                 start=(ac == 0), stop=(ac == KC - 1))
            rden = pp.tile([1, JW], f32, name="rden")
            nc.vector.reciprocal(rden[:], dps[:, 0:JW])
            rbc_ps = psB.tile([P, F], f32, tag="psB", name="rbc_ps")
            nc.tensor.matmul(rbc_ps[:, 0:JW], ones_rf[:], rden[:],
                             start=True, stop=True)
            rbc = pp.tile([P, JW], f32, name="rbc")
            nc.vector.tensor_copy(rbc[:], rbc_ps[:, 0:JW])

            # ---- W2 = Wv^T E diag(1/den), r = bv^T E diag(1/den) ----
            for dc in range(KC):
                wps = psB.tile([P, F], f32, tag="psB", name="wps")
                for ac in range(KC):
                    nc.tensor.matmul(wps[:, 0:JW],
                                     wvs[:, ac, dc * P:(dc + 1) * P],
                                     e_sb[:, ac, :],
                                     start=(ac == 0), stop=(ac == KC - 1))
                w2st = sp.tile([P, JW], bf16, tag="w2st", name="w2st")
                nc.vector.tensor_tensor(w2st[:], wps[:, 0:JW], rbc[:], mult)
                nc.sync.dma_start(ag_in[dc * P:(dc + 1) * P, :], w2st[:])
            rps = psC.tile([1, F], f32, tag="psC", name="rps")
            for ac in range(KC):
                nc.tensor.matmul(rps[:, 0:JW], bvs[:, ac:ac + 1],
                                 e_sb[:, ac, :],
                                 start=(ac == 0), stop=(ac == KC - 1))
            rst = sp.tile([1, JW], bf16, tag="rst", name="rst")
            nc.vector.tensor_tensor(rst[:], rps[:, 0:JW], rden[:], mult)
            nc.sync.dma_start(ag_in[D:D + 1, :], rst[:])
            nc.gpsimd.collective_compute(
                "AllGather", mybir.AluOpType.bypass, replica_groups=RG,
                ins=[ag_in.opt()], outs=[ag_out.opt()])

            # ---- out = x @ W2 + 1 r ----
            w2full = pp.tile([P, KC, D], bf16, name="w2full")
            for cc in range(NCORES):
                nc.sync.dma_start(
                    w2full[:, :, cc * JW:(cc + 1) * JW],
                    ag_out[cc, 0:D, :].rearrange("(c p) j -> p c j", p=P))
            r_row = pp.tile([1, D], bf16, name="r_row")
            nc.sync.dma_start(
                r_row[:].rearrange("o (c j) -> o c j", c=NCORES),
                ag_out[:, D:D + 1, :].transpose([1, 0, 2]))
            rb_sb = pp.tile([P, D], f32, name="rb_sb")
            for h in range(2):
                rbp = psB.tile([P, F], f32, tag="psB", name="rbp")
                nc.tensor.matmul(rbp[:], ones_rb[:], r_row[:, h * F:(h + 1) * F],
                                 start=True, stop=True)
                nc.vector.tensor_copy(rb_sb[:, h * F:(h + 1) * F], rbp[:])
            for h in range(2):
                for nch in range(KC):
                    ops = psA.tile([P, F], f32, tag="psA", name="ops")
                    for dc in range(KC):
                        nc.tensor.matmul(ops[:],
                                         xts[:, dc, nch * P:(nch + 1) * P],
                                         w2full[:, dc, h * F:(h + 1) * F],
                                         start=(dc == 0), stop=(dc == KC - 1))
                    ost = sp.tile([P, F], f32, tag="ost", name="ost")
                    nc.vector.tensor_tensor(ost[:], ops[:],
                                            rb_sb[:, h * F:(h + 1) * F], add)
                    nc.sync.dma_start(
                        out[nch * P:(nch + 1) * P, h * F:(h + 1) * F], ost[:])

    nc.compile()
    return nc


def _prep_inputs(x, Wq, bq, Wk, bk, Wv, bv):
    bf16 = ml_dtypes.bfloat16
    x = np.asarray(x, np.float32)
    xb = x.astype(bf16)
    wqT = np.ascontiguousarray(np.asarray(Wq).astype(bf16).T)
    wkT = np.ascontiguousarray(np.asarray(Wk).astype(bf16).T)
    wvn = np.ascontiguousarray(np.asarray(Wv).astype(bf16))
    bq_r = np.asarray(bq, np.float32).reshape(1, D).astype(bf16)
    bv_c = np.ascontiguousarray(
        np.asarray(bv, np.float32).astype(bf16).reshape(KC, P).T)
    in_maps = []
    for c in range(NCORES):
        shard = np.ascontiguousarray(xb[c * NPC:(c + 1) * NPC, :])
        shardT = np.ascontiguousarray(shard.T)
        jc = slice(c * JW, (c + 1) * JW)
        bk_jc = np.asarray(bk, np.float32)[jc]
        in_maps.append({
            "xn": shard, "xt": shardT, "wqT": wqT, "wvn": wvn,
            "wkj": np.ascontiguousarray(wkT[:, jc]),
            "bq_r": bq_r,
            "bk_r": bk_jc.reshape(1, JW).astype(bf16),
            "bkN_r": np.ascontiguousarray(
                (float(N) * bk_jc).reshape(1, JW).astype(np.float32)),
            "bv_c": bv_c,
        })
    return in_maps


def _ensure_axon_hooks_stub():
    # bass_utils imports antenv.axon_hooks when tracing is requested; this
    # image ships antenv without that submodule, so stub it to degrade
    # gracefully.
    import sys
    import types
    try:
        import antenv.axon_hooks  # noqa: F401
        return
    except ImportError:
        pass
    mod = types.ModuleType("antenv.axon_hooks")
    mod._hook = None
    mod.set_axon_ntff_profile_hook = lambda h: setattr(mod, "_hook", h)
    mod.get_axon_ntff_profile_hook = lambda: mod._hook
    sys.modules["antenv.axon_hooks"] = mod
    try:
        import antenv
        antenv.axon_hooks = mod
    except ImportError:
        pass


def kernel(x, Wq, bq, Wk, bk, Wv, bv, _trace=False):
    from concourse import bass_utils

    _ensure_axon_hooks_stub()

    if "nc" not in _cache:
        _cache["nc"] = _build_nc()
    nc = _cache["nc"]

    in_maps = _prep_inputs(x, Wq, bq, Wk, bk, Wv, bv)
    res = bass_utils.run_bass_kernel_spmd(
        nc, in_maps, core_ids=list(range(NCORES)), trace=_trace)
    _cache["last_result"] = res
    return np.concatenate(
        [res.results[c]["out"] for c in range(NCORES)], axis=0)
